# revision 1
# baseline (speedup 1.0000x reference)
"""BiMapGeo forward on 8 NeuronCores (TRN2, Bass/Tile).

P[b,o] = sum_c W[o,c]^T @ x[b,c] @ W[o,c]
  x: (256, 8, 128, 128) fp32 (symmetric in last two dims)
  W: (16, 8, 128, 64) fp32
  P: (256, 16, 64, 64) fp32

Sharding: data-parallel over batch (32 per core), W replicated.

Per-core kernel structure (per group of BG=4 batches):
  mm1: M1[b,c] = x[b,c] @ Wc           (fp32r, stationary=x[b,c] via symmetry,
                                        moving=W[:,c] as [128, 1024] in 2x512)
  evict: PSUM fp32 -> SBUF bf16        (VectorE / ScalarE alternating)
  mm2: P[b,o] += W[o,c]^T @ M1[b,o,c]  (bf16, col-tiled o-pairs, N=256,
                                        accumulate over c in PSUM fp32)
  evict P -> SBUF -> DMA out
"""

import numpy as np
from contextlib import ExitStack

import concourse.bacc as bacc
import concourse.tile as tile
from concourse import mybir

B_TOT, HI, HO, NI, NO = 256, 8, 16, 128, 64
NCORES = 8
B = B_TOT // NCORES  # 32 batches per core
BG = 4               # batches per group
NG = B // BG         # 8 groups
OQ = HO * NO         # 1024

F32 = mybir.dt.float32
F32R = mybir.dt.float32r
BF16 = mybir.dt.bfloat16

_NC_CACHE = {}


def build_nc(loop_iters: int = 1):
    nc = bacc.Bacc("TRN2", target_bir_lowering=False, debug=False)

    x_in = nc.dram_tensor("x", [B, HI, NI, NI], F32, kind="ExternalInput")
    w_in = nc.dram_tensor("W", [HO, HI, NI, NO], F32, kind="ExternalInput")
    p_out = nc.dram_tensor("P", [B, HO, NO, NO], F32, kind="ExternalOutput")

    with tile.TileContext(nc) as tc, ExitStack() as ctx:
        const = ctx.enter_context(tc.tile_pool(name="const", bufs=1))
        wstage = ctx.enter_context(tc.tile_pool(name="wstage", bufs=1))
        xstage = ctx.enter_context(tc.tile_pool(name="xstage", bufs=4))
        xpool = ctx.enter_context(tc.tile_pool(name="xpool", bufs=2))
        m1pool = ctx.enter_context(tc.tile_pool(name="m1pool", bufs=11))
        ppool = ctx.enter_context(tc.tile_pool(name="ppool", bufs=2))
        m1ps_pool = ctx.enter_context(tc.tile_pool(name="m1ps", bufs=3, space="PSUM"))
        pps_pool = ctx.enter_context(tc.tile_pool(name="pps", bufs=2, space="PSUM"))

        # W resident in SBUF as [j(128), c, o, q]: fp32r for mm1 moving operand,
        # bf16 for mm2 stationary operand. fp32r tiles must be produced by a
        # compute-engine rounding copy (walrus verifier), so DMA into a staging
        # tile and round into dedicated fp32r tiles.
        w_f32r = const.tile([NI, HI, HO, NO], F32R, tag="w_f32r")
        w_bf = const.tile([NI, HI, HO, NO], BF16, tag="w_bf")
        for c in range(HI):
            w_st = wstage.tile([NI, HO, NO], F32, tag="wst", name=f"wst{c}")
            nc.scalar.dma_start(out=w_st[:], in_=w_in[:, c, :, :].transpose([1, 0, 2]))
            nc.vector.tensor_copy(w_bf[:, c], w_st[:])
            nc.vector.tensor_copy(w_f32r[:, c], w_st[:])

        def emit_body():
            emit_groups(nc, tc, x_in, p_out, w_f32r, w_bf, xstage, xpool, m1pool, ppool, m1ps_pool, pps_pool)

        if loop_iters > 1:
            ET = mybir.EngineType
            with tc.For_i(0, loop_iters, 1, hint_engines=(ET.PE, ET.DVE, ET.Activation, ET.SP)):
                emit_body()
        else:
            emit_body()
    nc.finalize()
    return nc


def emit_groups(nc, tc, x_in, p_out, w_f32r, w_bf, xstage, xpool, m1pool, ppool, m1ps_pool, pps_pool):
        def mm2(pps_t, c, o, ph, m1_c):
            nc.tensor.matmul(
                pps_t[ph * 64 : (ph + 1) * 64, :],
                w_bf[:, c, o, :],
                m1_c[:, :, o * 64 : (o + 1) * 64],
                start=(c == 0),
                stop=(c == HI - 1),
                tile_position=(0, ph * 64),
                skip_group_check=True,
            )

        def evict_wave(g, wv, pps, b0):
            # 2 banks -> one SBUF tile -> one DMA per batch for o-pairs 2wv,2wv+1
            p_sb = ppool.tile([NI, 2, BG, NO], F32, tag="psb", name=f"psb_g{g}w{wv}")
            for t in range(2):
                nc.scalar.copy(p_sb[:, t, :, :], pps[t][:])
            for b in range(BG):
                nc.gpsimd.dma_start(
                    out=p_out[b0 + b, 4 * wv : 4 * wv + 4].rearrange(
                        "(t ph) p q -> ph p t q", ph=2
                    ),
                    in_=p_sb[:, :, b, :],
                )

        # x tile: [i(128), b, c, j]; by symmetry also usable as [j, b, c, i].
        # DMAs are emitted one group ahead so they sit ahead of the P-output
        # DMAs in the SP queue (avoids head-of-line blocking).
        def x_load(g):
            # per-batch DMA + round so mm1 can start after 1/4 of the transfer
            x_t = xpool.tile([NI, BG, HI, NI], F32R, tag="xt", name=f"xt{g}")
            for b in range(BG):
                x_sb = xstage.tile([NI, HI, NI], F32, tag="xst", name=f"xst{g}b{b}")
                nc.sync.dma_start(
                    out=x_sb[:], in_=x_in[g * BG + b].transpose([1, 0, 2])
                )
                nc.gpsimd.tensor_copy(x_t[:, b], x_sb[:])
            return x_t

        x_tiles = {0: x_load(0)}

        for g in range(NG):
            b0 = g * BG
            if g + 1 < NG:
                x_tiles[g + 1] = x_load(g + 1)
            x_t = x_tiles.pop(g)

            # wave A (o-pairs 0,1) PSUM accumulators, held across the c loop
            ppsA = [
                pps_pool.tile([NI, BG * NO], F32, tag="pps", name=f"ppsA_g{g}t{t}")
                for t in range(2)
            ]

            # mm1 + eviction + wave-A mm2, c-granular so everything pipelines
            m1_tiles = []
            for c in range(HI):
                m1_c = m1pool.tile([NI, BG, OQ], BF16, tag="m1")
                m1_tiles.append(m1_c)
                for b in range(BG):
                    m1_ps = m1ps_pool.tile([NI, OQ], F32, tag="m1ps")
                    lhsT = x_t[:, b, c, :]
                    for h in range(2):
                        nc.tensor.matmul(
                            m1_ps[:, h * 512 : (h + 1) * 512],
                            lhsT,
                            w_f32r[:, c, h * 8 : (h + 1) * 8, :],
                            start=True,
                            stop=True,
                        )
                    if (c * BG + b) % 2 == 0:
                        nc.vector.tensor_copy(m1_c[:, b, :], m1_ps[:, :])
                    else:
                        nc.scalar.copy(m1_c[:, b, :], m1_ps[:, :])
                    # software pipelining: wave-A mm2 of the PREVIOUS c,
                    # interleaved between mm1 pairs to fill eviction latency
                    if c > 0 and b < 2:
                        t = b
                        for ph in range(2):
                            mm2(ppsA[t], c - 1, 2 * t + ph, ph, m1_tiles[c - 1])
            for t in range(2):
                for ph in range(2):
                    mm2(ppsA[t], HI - 1, 2 * t + ph, ph, m1_tiles[HI - 1])

            evict_wave(g, 0, ppsA, b0)

            # post waves (o-pairs 2..7): o-outer / c-inner dense PE tails
            for wv in range(3):
                ppsB = [
                    pps_pool.tile([NI, BG * NO], F32, tag="pps", name=f"ppsB_g{g}w{wv}t{t}")
                    for t in range(2)
                ]
                for t in range(2):
                    wp = 2 + wv * 2 + t
                    for c in range(HI):
                        for ph in range(2):
                            mm2(ppsB[t], c, 2 * wp + ph, ph, m1_tiles[c])
                evict_wave(g, 1 + wv, ppsB, b0)


def kernel(x: np.ndarray, W: np.ndarray) -> np.ndarray:
    from concourse.bass_utils import run_bass_kernel_spmd

    x = np.ascontiguousarray(x, dtype=np.float32)
    W = np.ascontiguousarray(W, dtype=np.float32)

    if "nc" not in _NC_CACHE:
        _NC_CACHE["nc"] = build_nc()
    nc = _NC_CACHE["nc"]

    in_maps = [
        {"x": x[i * B : (i + 1) * B], "W": W} for i in range(NCORES)
    ]
    res = run_bass_kernel_spmd(nc, in_maps, list(range(NCORES)))
    out = np.concatenate([res.results[i]["P"] for i in range(NCORES)], axis=0)
    return out



# revision 3
# speedup vs baseline: 2.7457x; 2.7457x over previous
"""BiMapGeo forward on 8 NeuronCores (TRN2, Bass/Tile).

P[b,o] = sum_c W[o,c]^T @ x[b,c] @ W[o,c]
  x: (256, 8, 128, 128) fp32 (symmetric in last two dims)
  W: (16, 8, 128, 64) fp32
  P: (256, 16, 64, 64) fp32

Sharding: data-parallel over batch (32 per core), W replicated.

Host casts x and W to bf16 (error ~3e-3 << 2e-2 budget), which halves the
input DMA and removes all on-chip dtype staging (no fp32r round copies).

Per-core kernel structure (per group of BG=4 batches):
  mm1: M1[b,c] = x[b,c] @ Wc           (bf16, stationary=x[b,c] via symmetry,
                                        moving=W[:,c] as [128, 1024] in 2x512)
  evict: PSUM fp32 -> SBUF bf16        (VectorE / ScalarE, balanced)
  mm2: P[b,o] += W[o,c]^T @ M1[b,o,c]  (bf16, col-tiled o-pairs, N=256,
                                        accumulate over c in PSUM fp32)
  evict P -> SBUF -> DMA out
"""

import numpy as np
from contextlib import ExitStack

import concourse.bacc as bacc
import concourse.tile as tile
from concourse import mybir

B_TOT, HI, HO, NI, NO = 256, 8, 16, 128, 64
NCORES = 8
B = B_TOT // NCORES  # 32 batches per core
BG = 4               # batches per group
NG = B // BG         # 8 groups
OQ = HO * NO         # 1024

F32 = mybir.dt.float32
BF16 = mybir.dt.bfloat16

_NC_CACHE = {}


def build_nc(loop_iters: int = 1):
    nc = bacc.Bacc("TRN2", target_bir_lowering=False, debug=False)

    x_in = nc.dram_tensor("x", [B, HI, NI, NI], BF16, kind="ExternalInput")
    w_in = nc.dram_tensor("W", [HO, HI, NI, NO], BF16, kind="ExternalInput")
    p_out = nc.dram_tensor("P", [B, HO, NO, NO], F32, kind="ExternalOutput")

    with tile.TileContext(nc) as tc, ExitStack() as ctx:
        const = ctx.enter_context(tc.tile_pool(name="const", bufs=1))
        xpool = ctx.enter_context(tc.tile_pool(name="xpool", bufs=2))
        m1pool = ctx.enter_context(tc.tile_pool(name="m1pool", bufs=11))
        ppool = ctx.enter_context(tc.tile_pool(name="ppool", bufs=2))
        m1ps_pool = ctx.enter_context(tc.tile_pool(name="m1ps", bufs=3, space="PSUM"))
        pps_pool = ctx.enter_context(tc.tile_pool(name="pps", bufs=2, space="PSUM"))

        # W resident in SBUF as [i(128), c, o, q] bf16; serves both as mm1
        # moving operand (W[j, oq] per c) and mm2 stationary (W[i, p] per o,c).
        w_sb = const.tile([NI, HI, HO, NO], BF16, tag="w_sb")
        for c in range(HI):
            nc.scalar.dma_start(
                out=w_sb[:, c], in_=w_in[:, c, :, :].transpose([1, 0, 2])
            )

        def emit_body():
            emit_groups(nc, tc, x_in, p_out, w_sb, xpool, m1pool, ppool, m1ps_pool, pps_pool)

        if loop_iters > 1:
            ET = mybir.EngineType
            with tc.For_i(0, loop_iters, 1, hint_engines=(ET.PE, ET.DVE, ET.Activation, ET.SP)):
                emit_body()
        else:
            emit_body()
    nc.finalize()
    return nc


def emit_groups(nc, tc, x_in, p_out, w_sb, xpool, m1pool, ppool, m1ps_pool, pps_pool):
        def mm2(pps_t, c, o, ph, m1_c):
            nc.tensor.matmul(
                pps_t[ph * 64 : (ph + 1) * 64, :],
                w_sb[:, c, o, :],
                m1_c[:, :, o * 64 : (o + 1) * 64],
                start=(c == 0),
                stop=(c == HI - 1),
                tile_position=(0, ph * 64),
                skip_group_check=True,
            )

        def evict_wave(g, wv, pps, b0):
            # 2 banks -> one SBUF tile -> one DMA per batch for o-pairs 2wv,2wv+1
            p_sb = ppool.tile([NI, 2, BG, NO], F32, tag="psb", name=f"psb_g{g}w{wv}")
            for t in range(2):
                if wv % 2 == 0:
                    nc.scalar.copy(p_sb[:, t, :, :], pps[t][:])
                else:
                    nc.vector.tensor_copy(p_sb[:, t, :, :], pps[t][:])
            for b in range(BG):
                nc.gpsimd.dma_start(
                    out=p_out[b0 + b, 4 * wv : 4 * wv + 4].rearrange(
                        "(t ph) p q -> ph p t q", ph=2
                    ),
                    in_=p_sb[:, :, b, :],
                )

        # x tile: [i(128), b, c, j] bf16; by symmetry also usable as [j, b, c, i].
        # DMAs are emitted one group ahead so they sit ahead of the P-output
        # DMAs in the SP queue (avoids head-of-line blocking).
        def x_load(g):
            x_t = xpool.tile([NI, BG, HI, NI], BF16, tag="xt", name=f"xt{g}")
            for b in range(BG):
                nc.sync.dma_start(
                    out=x_t[:, b], in_=x_in[g * BG + b].transpose([1, 0, 2])
                )
            return x_t

        x_tiles = {0: x_load(0)}

        for g in range(NG):
            b0 = g * BG
            if g + 1 < NG:
                x_tiles[g + 1] = x_load(g + 1)
            x_t = x_tiles.pop(g)

            # wave A (o-pairs 0,1) PSUM accumulators, held across the c loop
            ppsA = [
                pps_pool.tile([NI, BG * NO], F32, tag="pps", name=f"ppsA_g{g}t{t}")
                for t in range(2)
            ]

            # mm1 + eviction + wave-A mm2, c-granular so everything pipelines
            m1_tiles = []
            for c in range(HI):
                m1_c = m1pool.tile([NI, BG, OQ], BF16, tag="m1")
                m1_tiles.append(m1_c)
                for b in range(BG):
                    m1_ps = m1ps_pool.tile([NI, OQ], F32, tag="m1ps")
                    lhsT = x_t[:, b, c, :]
                    for h in range(2):
                        nc.tensor.matmul(
                            m1_ps[:, h * 512 : (h + 1) * 512],
                            lhsT,
                            w_sb[:, c, h * 8 : (h + 1) * 8, :],
                            start=True,
                            stop=True,
                        )
                    k = c * BG + b
                    # ScalarE is slightly faster per PSUM copy; give it 17/32
                    if k == 0 or k % 2 == 1:
                        nc.scalar.copy(m1_c[:, b, :], m1_ps[:, :])
                    else:
                        nc.vector.tensor_copy(m1_c[:, b, :], m1_ps[:, :])
                    # software pipelining: wave-A mm2 of the PREVIOUS c,
                    # interleaved between mm1 pairs to fill eviction latency
                    if c > 0 and b < 2:
                        t = b
                        for ph in range(2):
                            mm2(ppsA[t], c - 1, 2 * t + ph, ph, m1_tiles[c - 1])
            for t in range(2):
                for ph in range(2):
                    mm2(ppsA[t], HI - 1, 2 * t + ph, ph, m1_tiles[HI - 1])

            evict_wave(g, 0, ppsA, b0)

            # post waves (o-pairs 2..7): o-outer / c-inner dense PE tails
            for wv in range(3):
                ppsB = [
                    pps_pool.tile([NI, BG * NO], F32, tag="pps", name=f"ppsB_g{g}w{wv}t{t}")
                    for t in range(2)
                ]
                for t in range(2):
                    wp = 2 + wv * 2 + t
                    for c in range(HI):
                        for ph in range(2):
                            mm2(ppsB[t], c, 2 * wp + ph, ph, m1_tiles[c])
                evict_wave(g, 1 + wv, ppsB, b0)


def kernel(x: np.ndarray, W: np.ndarray) -> np.ndarray:
    import ml_dtypes
    from concourse.bass_utils import run_bass_kernel_spmd

    x = np.ascontiguousarray(x.astype(ml_dtypes.bfloat16))
    W = np.ascontiguousarray(W.astype(ml_dtypes.bfloat16))

    if "nc" not in _NC_CACHE:
        _NC_CACHE["nc"] = build_nc()
    nc = _NC_CACHE["nc"]

    in_maps = [
        {"x": x[i * B : (i + 1) * B], "W": W} for i in range(NCORES)
    ]
    res = run_bass_kernel_spmd(nc, in_maps, list(range(NCORES)))
    out = np.concatenate([res.results[i]["P"] for i in range(NCORES)], axis=0)
    return out
